# revision 6
# baseline (speedup 1.0000x reference)
"""Trainium2 Bass kernel for nn_AttributeOperator (MoE-style routing).

Computes out[b] = relu(attr_ops[attrs[b]] @ obj_emb[objs[b]]) for b in [0, B).

Strategy (expert-parallel): the dominant cost is streaming the attr_ops table
(N_ATTRS x D x D fp32 = 512 MB). Samples are grouped by attribute on the host,
groups are load-balanced across the 8 cores (snake deal by group size), and
each core streams only its own subset of operator matrices from HBM exactly
once, cast to fp16 on the host (halves the stream; max abs err ~6e-4 vs the
f32 reference, resid_var ~1e-7). Per group the core computes X @ A^T on
TensorE (X^T stationary, A^T streaming at N=512) accumulating in f32 PSUM
over the 4 K-chunks of 128, applies
ReLU on ScalarE and DMAs the rows out from the same engine (no cross-engine
hop). The matrix stream owns the sync HWDGE ring exclusively; outputs use the
scalar ring. The host scatters rows back to their original batch positions.

attr_ops matrices are pre-transposed on the host so the contraction dim (j) is
the SBUF partition dim, making the device DMA fully contiguous. The SPMD
program is identical on all 8 cores; only the per-core input tensors differ.
Slot s has a fixed column capacity maxc[s] = max over cores of that rank's
group size, so the one program fits every core's routing.
"""

import numpy as np
import ml_dtypes

import concourse.tile as tile
from concourse import bacc, mybir
from concourse.bass_utils import run_bass_kernel_spmd

N_CORES = 8
D = 512               # embedding dim (hardcoded per problem spec)
QCH = D // 128        # contraction chunks of 128 partitions

# attr_ops are shipped as fp8-e3m4 scaled by OPS_SCALE (values ~N(0, 0.02);
# unscaled they'd all land in e3m4's subnormal range). The 1/OPS_SCALE dequant
# is folded into the fp16 xt on the host, so the device sees plain matmuls.
# Measured rel-err vs the f32 reference: 1.39e-2 (gate: 2e-2); e4m3 fails at
# 2.3e-2, so DoubleRow (fp8e4/e5-only) is not available.
OPS_SCALE = 64.0

# test.py hooks (ignored by the grading harness)
LAST_RESULTS = None   # BassKernelResults of the most recent run
TRACE = False
TRACE_CORES = None

PAIR = 4
_NC_CACHE = {}


def _build_nc(maxc, offs, ncol, ops_bufs=8, pair=None, sync_frac=(1, 1), reps=1,
              out_engine="scalar", staggered=False, relu_engine="scalar",
              xt_engine="scalar"):
    """Build + compile the SPMD program.

    maxc[s]: column capacity of slot s; offs[s]: column offset of slot s;
    ncol: total columns (= offs[-1] + maxc[-1]).
    pair: matrices loaded per ops DMA (amortizes per-DMA fixed costs).
    sync_frac: (a, b) -> a of every b ops DMAs issue on sync, rest on scalar.
    reps: hardware-loop repetitions of the whole kernel (for timing).
    staggered: staggered-reset loop back-edge — wedges this device, keep False.
    """
    if pair is None:
        pair = PAIR
    nm = len(maxc)
    nmp = -(-nm // pair) * pair  # nm rounded up to a multiple of pair
    ng = nmp // pair
    nc = bacc.Bacc("TRN2", target_bir_lowering=False, debug=False,
                   num_devices=N_CORES)
    # per-group layout [p, t, q, i]: each partition's data is one contiguous
    # pair*QCH*D-byte run -> one big DMA descriptor per partition
    ops_dram = nc.dram_tensor("ops_t", [ng, 128, pair * QCH * D],
                              mybir.dt.float8e3, kind="ExternalInput").ap()
    xt_dram = nc.dram_tensor("xt", [128, QCH * ncol], mybir.dt.float16,
                             kind="ExternalInput").ap()
    out_dram = nc.dram_tensor("out", [ncol, D], mybir.dt.float32,
                              kind="ExternalOutput").ap()

    with tile.TileContext(nc) as tc:
        with (
            tc.tile_pool(name="xt", bufs=1) as xt_pool,
            tc.tile_pool(name="ops", bufs=ops_bufs) as ops_pool,
            tc.tile_pool(name="ps", bufs=8, space="PSUM") as ps_pool,
            tc.tile_pool(name="o", bufs=4) as o_pool,
        ):
            def body():
                xt_sb = xt_pool.tile([128, QCH * ncol], mybir.dt.float16)
                getattr(nc, xt_engine).dma_start(xt_sb[:], xt_dram[:])

                for g in range(ng):
                    m = ops_pool.tile([128, pair * QCH * D],
                                      mybir.dt.float8e3, tag="m")
                    issuer = nc.sync if g % sync_frac[1] < sync_frac[0] \
                        else nc.scalar
                    issuer.dma_start(m[:], ops_dram[g])
                    for t in range(pair):
                        s = g * pair + t
                        if s >= nm:
                            break
                        cw = maxc[s]
                        ps = ps_pool.tile([cw, D], mybir.dt.float32, tag="ps")
                        for q in range(QCH):
                            lhsT = xt_sb[:, q * ncol + offs[s]:
                                         q * ncol + offs[s] + cw]
                            rhs = m[:, (t * QCH + q) * D:
                                    (t * QCH + q + 1) * D]
                            nc.tensor.matmul(ps[:], lhsT, rhs,
                                             start=(q == 0),
                                             stop=(q == QCH - 1))
                        o = o_pool.tile([cw, D], mybir.dt.float32, tag="o")
                        if relu_engine == "vector":
                            nc.vector.tensor_scalar_max(o[:], ps[:], 0.0)
                        else:
                            nc.scalar.activation(
                                o[:], ps[:], mybir.ActivationFunctionType.Relu)
                        out_eng = getattr(nc, out_engine)
                        out_eng.dma_start(
                            out_dram[offs[s]:offs[s] + cw, :], o[:])

            if reps == 1:
                body()
            else:
                with tc.For_i(0, reps, 1,
                              hint_engines=(mybir.EngineType.PE,),
                              staggered_reset=staggered):
                    body()

    nc.compile()
    return nc


def _route(attrs):
    """Group sample indices by attribute, chunk to <=128, snake-balance
    across cores. Returns per-core slot lists of (attr_id, idx_array),
    each list sorted by descending group size."""
    order = np.argsort(attrs, kind="stable")
    sorted_attrs = attrs[order]
    uniq, starts, counts = np.unique(sorted_attrs, return_index=True,
                                     return_counts=True)
    chunks = []
    for a, st, c in zip(uniq, starts, counts):
        idx = order[st:st + c]
        for o in range(0, c, 128):
            chunks.append((int(a), idx[o:o + 128]))
    chunks.sort(key=lambda t: -len(t[1]))
    per_core = [[] for _ in range(N_CORES)]
    for i, ch in enumerate(chunks):
        r, pos = divmod(i, N_CORES)
        k = pos if r % 2 == 0 else N_CORES - 1 - pos
        per_core[k].append(ch)
    return per_core


def _layout(per_core):
    """Per-slot-rank column capacity/offset shared by all cores."""
    nm = max(1, max(len(s) for s in per_core))
    maxc = [1] * nm
    for slots in per_core:
        for s, (_, idx) in enumerate(slots):
            maxc[s] = max(maxc[s], len(idx))
    offs = [0] * nm
    for s in range(1, nm):
        offs[s] = offs[s - 1] + maxc[s - 1]
    ncol = offs[-1] + maxc[-1]
    return nm, maxc, offs, ncol


def _prepare(attrs, objs, attr_ops, obj_emb):
    """Route + build per-core device input maps."""
    per_core = _route(attrs)
    nm, maxc, offs, ncol = _layout(per_core)
    nmp = -(-nm // PAIR) * PAIR

    rep = obj_emb[objs] * np.float32(1.0 / OPS_SCALE)  # [B, D] object reps
    ng = nmp // PAIR
    in_maps = []
    for k in range(N_CORES):
        slots = per_core[k]
        # ops_t[g, p, (t, q, i)] = OPS_SCALE * A_s[i, q*128 + p], s = g*PAIR+t
        ops_t = np.zeros((ng, 128, PAIR, QCH, D), ml_dtypes.float8_e3m4)
        r = np.zeros((ncol, D), np.float32)
        for s, (a, idx) in enumerate(slots):
            g, t = divmod(s, PAIR)
            ops_t[g, :, t] = (attr_ops[a].T * np.float32(OPS_SCALE)).reshape(
                QCH, 128, D).transpose(1, 0, 2)
            r[offs[s]:offs[s] + len(idx)] = rep[idx]
        # xt[p, q*ncol + c] = r[c, q*128 + p]
        xt = np.ascontiguousarray(r.reshape(ncol, QCH, 128).transpose(
            2, 1, 0).astype(np.float16)).reshape(128, -1)
        in_maps.append({"ops_t": ops_t.reshape(ng, 128, PAIR * QCH * D),
                        "xt": xt})
    return per_core, (nm, tuple(maxc), tuple(offs), ncol), in_maps


def kernel(attrs, objs, attr_ops, obj_emb):
    global LAST_RESULTS
    attrs = np.asarray(attrs)
    objs = np.asarray(objs)
    attr_ops = np.asarray(attr_ops, dtype=np.float32)
    obj_emb = np.asarray(obj_emb, dtype=np.float32)
    B = attrs.shape[0]
    d = obj_emb.shape[1]
    assert d == D and attr_ops.shape[1:] == (D, D)

    per_core, (nm, maxc, offs, ncol), in_maps = _prepare(
        attrs, objs, attr_ops, obj_emb)

    nc = _NC_CACHE.get(maxc)
    if nc is None:
        nc = _NC_CACHE[maxc] = _build_nc(maxc, offs, ncol, pair=PAIR)

    res = run_bass_kernel_spmd(nc, in_maps, core_ids=list(range(N_CORES)),
                               trace=TRACE, trace_cores=TRACE_CORES)
    LAST_RESULTS = res

    out = np.zeros((B, d), np.float32)
    for k in range(N_CORES):
        out_k = res.results[k]["out"]
        for s, (a, idx) in enumerate(per_core[k]):
            out[idx] = out_k[offs[s]:offs[s] + len(idx)]
    return out



# revision 7
# speedup vs baseline: 1.4535x; 1.4535x over previous
"""Trainium2 Bass kernel for nn_AttributeOperator (MoE-style routing).

Computes out[b] = relu(attr_ops[attrs[b]] @ obj_emb[objs[b]]) for b in [0, B).

Strategy (expert-parallel, fp8, column-tiled): the dominant cost is streaming
the attr_ops table (512 x 512 x 512 fp32 = 512 MB). Samples are grouped by
attribute on the host, groups are chunked to <= 32 samples, and chunks are
load-balanced across the 8 cores (snake deal by chunk size); each core streams
only its own subset of operator matrices from HBM exactly once, cast on the
host to fp8-e3m4 scaled by OPS_SCALE (the 1/OPS_SCALE dequant is folded into
the fp16 xt). e3m4's 4 mantissa bits give rel-err 1.39e-2 vs the f32
reference (gate 2e-2); e4m3 fails at 2.3e-2, which rules out DoubleRow.

Per core the work is 16 super-groups (sg) of 4 slots; slot (sg, j) owns a
fixed 32-sample capacity. The four slots of a super-group run CONCURRENTLY in
the four 32-column groups of the PE array via tile_position=(0, 32j), all
accumulating into one [128, 512] PSUM bank (slot j at partitions 32j..32j+31).
This 4x-overlaps the A-matrix streaming (the PE-time floor is the 1 col/cycle
moving-operand ingest) and lets one full-width [128, 512] relu + one 128 KB
out-DMA + one 1 MB ops-DMA serve 4 slots, keeping the serial HWDGE
descriptor-generation cost (~0.6 us per DMA) and the ACT relu cost off the
critical path. The matrix stream owns the sync HWDGE ring; xt/out use the
scalar ring. Output rows live at fixed padded positions (core, sg, 32j+c) in
fp16; the host scatters them back to batch order.

The SPMD program is identical on all 8 cores and input-independent (unused
slots stream zero matrices), so it compiles exactly once.
"""

import numpy as np
import ml_dtypes

import concourse.tile as tile
from concourse import bacc, mybir
from concourse.bass_utils import run_bass_kernel_spmd

N_CORES = 8
D = 512               # embedding dim (hardcoded per problem spec)
QCH = D // 128        # contraction chunks of 128 partitions
CW = 32               # column-tile width = per-slot sample capacity
NJ = 4                # concurrent col-tiled slots per super-group
SG = 16               # super-groups per core (SG*NJ slots of CW samples)

# attr_ops values are ~N(0, 0.02); unscaled they'd land in e3m4's subnormal
# range, so scale into the normal range and fold 1/OPS_SCALE into xt.
OPS_SCALE = 64.0

# test.py hooks (ignored by the grading harness)
LAST_RESULTS = None   # BassKernelResults of the most recent run
TRACE = False
TRACE_CORES = None

_NC_CACHE = {}


def _build_nc(sg=SG, ops_bufs=6, reps=1, staggered=False):
    """Build + compile the SPMD program (input-independent).

    sg: number of super-groups; reps: hardware-loop repetitions (timing).
    """
    nc = bacc.Bacc("TRN2", target_bir_lowering=False, debug=False,
                   num_devices=N_CORES)
    # ops[g, p, (j, q, i)] = OPS_SCALE * A_{g,j}[i, q*128 + p] in e3m4:
    # one contiguous NJ*QCH*D-byte run per partition -> 1 MB DMA per sg
    ops_dram = nc.dram_tensor("ops_t", [sg, 128, NJ * QCH * D],
                              mybir.dt.float8e3, kind="ExternalInput").ap()
    # xt[p, (q, g, j, c)] = X_{g,j}[c, q*128 + p] / OPS_SCALE
    xt_dram = nc.dram_tensor("xt", [128, QCH * sg * NJ * CW],
                             mybir.dt.float16, kind="ExternalInput").ap()
    # out[g, 32j + c, :] = relu(A_{g,j} @ X_{g,j}[c]) in fp16
    out_dram = nc.dram_tensor("out", [sg, 128, D], mybir.dt.float16,
                              kind="ExternalOutput").ap()

    with tile.TileContext(nc) as tc:
        with (
            tc.tile_pool(name="xt", bufs=1) as xt_pool,
            tc.tile_pool(name="ops", bufs=ops_bufs) as ops_pool,
            tc.tile_pool(name="ps", bufs=8, space="PSUM") as ps_pool,
            tc.tile_pool(name="o", bufs=4) as o_pool,
        ):
            def body():
                xt_sb = xt_pool.tile([128, QCH * sg * NJ * CW],
                                     mybir.dt.float16)
                nc.scalar.dma_start(xt_sb[:], xt_dram[:])

                for g in range(sg):
                    m = ops_pool.tile([128, NJ * QCH * D],
                                      mybir.dt.float8e3, tag="m")
                    nc.sync.dma_start(m[:], ops_dram[g])
                    ps = ps_pool.tile([128, D], mybir.dt.float32, tag="ps")
                    for q in range(QCH):
                        for j in range(NJ):
                            lhsT = xt_sb[:, ((q * sg + g) * NJ + j) * CW:
                                         ((q * sg + g) * NJ + j + 1) * CW]
                            rhs = m[:, (j * QCH + q) * D:
                                    (j * QCH + q + 1) * D]
                            nc.tensor.matmul(ps[32 * j:32 * (j + 1), :],
                                             lhsT, rhs,
                                             start=(q == 0),
                                             stop=(q == QCH - 1),
                                             tile_position=(0, 32 * j))
                    o = o_pool.tile([128, D], mybir.dt.float16, tag="o")
                    nc.scalar.activation(
                        o[:], ps[:], mybir.ActivationFunctionType.Relu)
                    nc.scalar.dma_start(out_dram[g], o[:])

            if reps == 1:
                body()
            else:
                with tc.For_i(0, reps, 1,
                              hint_engines=(mybir.EngineType.PE,),
                              staggered_reset=staggered):
                    body()

    nc.compile()
    return nc


def _route(attrs):
    """Group sample indices by attribute, chunk to <= CW, snake-balance
    across cores. Returns per-core slot lists of (attr_id, idx_array),
    sorted by descending chunk size."""
    order = np.argsort(attrs, kind="stable")
    sorted_attrs = attrs[order]
    uniq, starts, counts = np.unique(sorted_attrs, return_index=True,
                                     return_counts=True)
    chunks = []
    for a, st, c in zip(uniq, starts, counts):
        idx = order[st:st + c]
        for o in range(0, c, CW):
            chunks.append((int(a), idx[o:o + CW]))
    chunks.sort(key=lambda t: -len(t[1]))
    per_core = [[] for _ in range(N_CORES)]
    for i, ch in enumerate(chunks):
        r, pos = divmod(i, N_CORES)
        k = pos if r % 2 == 0 else N_CORES - 1 - pos
        per_core[k].append(ch)
    return per_core


def _quantize_ops(attr_ops, attr_ids):
    """e3m4-quantize A^T for the given attribute ids.

    Returns {attr_id: [QCH, 128, D] e3m4 array} = OPS_SCALE * A.T reshaped so
    [q, p, i] = A[i, q*128 + p]."""
    out = {}
    ids = np.asarray(sorted(attr_ids))
    for blk in range(0, len(ids), 32):
        b = ids[blk:blk + 32]
        at = np.ascontiguousarray(
            attr_ops[b].transpose(0, 2, 1)) * np.float32(OPS_SCALE)
        q8 = at.astype(ml_dtypes.float8_e3m4).reshape(-1, QCH, 128, D)
        for i, a in enumerate(b):
            out[int(a)] = q8[i]
    return out


def _prepare(attrs, objs, attr_ops, obj_emb):
    """Route + build per-core device input maps."""
    per_core = _route(attrs)
    nslots = max(len(s) for s in per_core)
    sg = max(SG, -(-nslots // NJ))

    rep = (obj_emb[objs] * np.float32(1.0 / OPS_SCALE)).astype(np.float16)
    q8 = _quantize_ops(attr_ops, {a for s in per_core for a, _ in s})
    in_maps = []
    for k in range(N_CORES):
        slots = per_core[k]
        ops_t = np.zeros((sg, 128, NJ, QCH, D), ml_dtypes.float8_e3m4)
        xt = np.zeros((128, QCH, sg, NJ, CW), np.float16)
        for s, (a, idx) in enumerate(slots):
            g, j = divmod(s, NJ)
            ops_t[g, :, j] = q8[a].transpose(1, 0, 2)
            # xt[p, q, g, j, c] = rep[idx[c], q*128 + p]
            xt[:, :, g, j, :len(idx)] = rep[idx].reshape(
                len(idx), QCH, 128).transpose(2, 1, 0)
        in_maps.append({"ops_t": ops_t.reshape(sg, 128, NJ * QCH * D),
                        "xt": xt.reshape(128, -1)})
    return per_core, sg, in_maps


def kernel(attrs, objs, attr_ops, obj_emb):
    global LAST_RESULTS
    attrs = np.asarray(attrs)
    objs = np.asarray(objs)
    attr_ops = np.asarray(attr_ops, dtype=np.float32)
    obj_emb = np.asarray(obj_emb, dtype=np.float32)
    B = attrs.shape[0]
    d = obj_emb.shape[1]
    assert d == D and attr_ops.shape[1:] == (D, D)

    per_core, sg, in_maps = _prepare(attrs, objs, attr_ops, obj_emb)

    nc = _NC_CACHE.get(sg)
    if nc is None:
        nc = _NC_CACHE[sg] = _build_nc(sg)

    res = run_bass_kernel_spmd(nc, in_maps, core_ids=list(range(N_CORES)),
                               trace=TRACE, trace_cores=TRACE_CORES)
    LAST_RESULTS = res

    out = np.zeros((B, d), np.float32)
    for k in range(N_CORES):
        out_k = res.results[k]["out"]  # [sg, 128, D] fp16
        for s, (a, idx) in enumerate(per_core[k]):
            g, j = divmod(s, NJ)
            out[idx] = out_k[g, 32 * j:32 * j + len(idx), :]
    return out


# revision 9
# speedup vs baseline: 1.6002x; 1.1009x over previous
"""Trainium2 Bass kernel for nn_AttributeOperator (MoE-style routing).

Computes out[b] = relu(attr_ops[attrs[b]] @ obj_emb[objs[b]]) for b in [0, B).

Strategy (expert-parallel, fp8, column-tiled): the dominant cost is streaming
the attr_ops table (512 x 512 x 512 fp32 = 512 MB). Samples are grouped by
attribute on the host, groups are chunked to <= 32 samples, and chunks are
load-balanced across the 8 cores (snake deal by chunk size); each core streams
only its own subset of operator matrices from HBM exactly once, cast on the
host to fp8-e3m4 scaled by OPS_SCALE (the 1/OPS_SCALE dequant is folded into
the fp16 xt). e3m4's 4 mantissa bits give rel-err 1.39e-2 vs the f32
reference (gate 2e-2); e4m3 fails at 2.3e-2, which rules out DoubleRow.

Per core the work is `sg` super-groups of 4 slots; slot rank s has a fixed
capacity cap[s] = max chunk size at that rank over all cores (SPMD: one
program fits every core's routing). The four slots of a super-group run
CONCURRENTLY in the four 32-column groups of the PE array via
tile_position=(0, 32*j), all accumulating into one [128, 512] PSUM bank (slot
j at partitions 32*j..32*j+cap). This 4x-overlaps the A-matrix streaming (the
PE-time floor is the 1 col/cycle moving-operand ingest) and lets one
full-width [128, 512] relu + one ops DMA serve 4 slots; out rows are relu'd
in fp16 into a shared buffer covering two super-groups per out-DMA. This
keeps the serial HWDGE descriptor-generation cost (~0.6 us per DMA) and the
ACT relu cost off the critical path; the ops stream owns the sync HWDGE ring
(xt/out use the scalar ring) and is the memory-roofline bottleneck.
"""

import numpy as np
import ml_dtypes

import concourse.tile as tile
from concourse import bacc, mybir
from concourse.bass_utils import run_bass_kernel_spmd

N_CORES = 8
D = 512               # embedding dim (hardcoded per problem spec)
QCH = D // 128        # contraction chunks of 128 partitions
CW = 32               # column-tile width = per-slot sample capacity
NJ = 4                # concurrent col-tiled slots per super-group
SG = 16               # minimum super-groups per core

# attr_ops values are ~N(0, 0.02); unscaled they'd land in e3m4's subnormal
# range, so scale into the normal range and fold 1/OPS_SCALE into xt.
OPS_SCALE = 64.0

# test.py hooks (ignored by the grading harness)
LAST_RESULTS = None   # BassKernelResults of the most recent run
TRACE = False
TRACE_CORES = None

OPG = 1               # super-groups per ops DMA
_NC_CACHE = {}


def _build_nc(caps, ops_bufs=8, opg=None, reps=1, staggered=False):
    """Build + compile the SPMD program.

    caps[s]: sample capacity of slot rank s (s = g*NJ + j), len divisible by
    2*NJ; opg: super-groups per ops DMA; reps: HW-loop repetitions (timing).
    """
    if opg is None:
        opg = OPG
    sg = len(caps) // NJ
    ct = sum(caps)
    coff = np.concatenate([[0], np.cumsum(caps)])
    ng = -(-sg // opg)
    nc = bacc.Bacc("TRN2", target_bir_lowering=False, debug=False,
                   num_devices=N_CORES)
    # ops[gg, p, (g', j, q, i)] = OPS_SCALE * A_{g,j}[i, q*128 + p] in e3m4:
    # one contiguous opg*NJ*QCH*D-byte run per partition per DMA
    ops_dram = nc.dram_tensor("ops_t", [ng, 128, opg * NJ * QCH * D],
                              mybir.dt.float8e3, kind="ExternalInput").ap()
    # xt[p, q*ct + coff[s] + c] = X_s[c, q*128 + p] / OPS_SCALE
    xt_dram = nc.dram_tensor("xt", [128, QCH * ct], mybir.dt.float16,
                             kind="ExternalInput").ap()
    # out[g2, 32j + c, half*512 + :] = relu(A_s @ X_s[c]) fp16, s=(2g2+half)*NJ+j
    out_dram = nc.dram_tensor("out", [sg // 2, 128, 2 * D], mybir.dt.float16,
                              kind="ExternalOutput").ap()

    with tile.TileContext(nc) as tc:
        with (
            tc.tile_pool(name="xt", bufs=1) as xt_pool,
            tc.tile_pool(name="ops", bufs=ops_bufs) as ops_pool,
            tc.tile_pool(name="ps", bufs=8, space="PSUM") as ps_pool,
            tc.tile_pool(name="o", bufs=4) as o_pool,
        ):
            def body():
                xt_sb = xt_pool.tile([128, QCH * ct], mybir.dt.float16)
                nc.scalar.dma_start(xt_sb[:], xt_dram[:])

                o = None
                for g in range(sg):
                    if g % opg == 0:
                        m = ops_pool.tile([128, opg * NJ * QCH * D],
                                          mybir.dt.float8e3, tag="m")
                        nc.sync.dma_start(m[:], ops_dram[g // opg])
                    ps = ps_pool.tile([128, D], mybir.dt.float32, tag="ps")
                    for q in range(QCH):
                        for j in range(NJ):
                            s = g * NJ + j
                            cw = caps[s]
                            lhsT = xt_sb[:, q * ct + coff[s]:
                                         q * ct + coff[s] + cw]
                            rhs = m[:, (((g % opg) * NJ + j) * QCH + q) * D:
                                    (((g % opg) * NJ + j) * QCH + q + 1) * D]
                            nc.tensor.matmul(ps[32 * j:32 * j + cw, :],
                                             lhsT, rhs,
                                             start=(q == 0),
                                             stop=(q == QCH - 1),
                                             tile_position=(0, 32 * j))
                    if g % 2 == 0:
                        o = o_pool.tile([128, 2 * D], mybir.dt.float16,
                                        tag="o")
                    nc.scalar.activation(
                        o[:, (g % 2) * D:(g % 2 + 1) * D], ps[:],
                        mybir.ActivationFunctionType.Relu)
                    if g % 2 == 1:
                        nc.scalar.dma_start(out_dram[g // 2], o[:])

            if reps == 1:
                body()
            else:
                with tc.For_i(0, reps, 1,
                              hint_engines=(mybir.EngineType.PE,),
                              staggered_reset=staggered):
                    body()

    nc.compile()
    return nc


def _route(attrs):
    """Group sample indices by attribute, chunk to <= CW, snake-balance
    across cores. Returns per-core slot lists of (attr_id, idx_array),
    sorted by descending chunk size."""
    order = np.argsort(attrs, kind="stable")
    sorted_attrs = attrs[order]
    uniq, starts, counts = np.unique(sorted_attrs, return_index=True,
                                     return_counts=True)
    chunks = []
    for a, st, c in zip(uniq, starts, counts):
        idx = order[st:st + c]
        for o in range(0, c, CW):
            chunks.append((int(a), idx[o:o + CW]))
    chunks.sort(key=lambda t: -len(t[1]))
    per_core = [[] for _ in range(N_CORES)]
    for i, ch in enumerate(chunks):
        r, pos = divmod(i, N_CORES)
        k = pos if r % 2 == 0 else N_CORES - 1 - pos
        per_core[k].append(ch)
    return per_core


def _quantize_ops(attr_ops, attr_ids):
    """e3m4-quantize A^T for the given attribute ids.

    Returns {attr_id: [QCH, 128, D] e3m4 array} with [q, p, i] =
    OPS_SCALE * A[i, q*128 + p]."""
    out = {}
    ids = np.asarray(sorted(attr_ids))
    for blk in range(0, len(ids), 32):
        b = ids[blk:blk + 32]
        at = np.ascontiguousarray(
            attr_ops[b].transpose(0, 2, 1)) * np.float32(OPS_SCALE)
        q8 = at.astype(ml_dtypes.float8_e3m4).reshape(-1, QCH, 128, D)
        for i, a in enumerate(b):
            out[int(a)] = q8[i]
    return out


def _layout(per_core):
    """Per-slot-rank capacities shared by all cores, padded to 2*NJ ranks."""
    nslots = max(1, max(len(s) for s in per_core))
    sg = max(SG, -(-nslots // NJ))
    sg += sg % 2
    caps = [1] * (sg * NJ)
    for slots in per_core:
        for s, (_, idx) in enumerate(slots):
            caps[s] = max(caps[s], len(idx))
    return tuple(caps)


def _prepare(attrs, objs, attr_ops, obj_emb):
    """Route + build per-core device input maps."""
    per_core = _route(attrs)
    caps = _layout(per_core)
    sg = len(caps) // NJ
    ng = -(-sg // OPG)
    ct = sum(caps)
    coff = np.concatenate([[0], np.cumsum(caps)])

    rep = (obj_emb[objs] * np.float32(1.0 / OPS_SCALE)).astype(np.float16)
    q8 = _quantize_ops(attr_ops, {a for s in per_core for a, _ in s})
    in_maps = []
    for k in range(N_CORES):
        slots = per_core[k]
        ops_t = np.zeros((ng * OPG, 128, NJ, QCH, D), ml_dtypes.float8_e3m4)
        xt = np.zeros((128, QCH, ct), np.float16)
        for s, (a, idx) in enumerate(slots):
            g, j = divmod(s, NJ)
            ops_t[g, :, j] = q8[a].transpose(1, 0, 2)
            # xt[p, q, coff[s] + c] = rep[idx[c], q*128 + p]
            xt[:, :, coff[s]:coff[s] + len(idx)] = rep[idx].reshape(
                len(idx), QCH, 128).transpose(2, 1, 0)
        # device layout [gg, p, (g', j, q, i)]: opg consecutive super-groups
        # concatenated along the free dim
        ops_dev = np.ascontiguousarray(
            ops_t.reshape(ng, OPG, 128, NJ * QCH * D).transpose(0, 2, 1, 3)
        ).reshape(ng, 128, OPG * NJ * QCH * D)
        in_maps.append({"ops_t": ops_dev, "xt": xt.reshape(128, -1)})
    return per_core, caps, in_maps


def kernel(attrs, objs, attr_ops, obj_emb):
    global LAST_RESULTS
    attrs = np.asarray(attrs)
    objs = np.asarray(objs)
    attr_ops = np.asarray(attr_ops, dtype=np.float32)
    obj_emb = np.asarray(obj_emb, dtype=np.float32)
    B = attrs.shape[0]
    d = obj_emb.shape[1]
    assert d == D and attr_ops.shape[1:] == (D, D)

    per_core, caps, in_maps = _prepare(attrs, objs, attr_ops, obj_emb)

    nc = _NC_CACHE.get(caps)
    if nc is None:
        nc = _NC_CACHE[caps] = _build_nc(caps)

    res = run_bass_kernel_spmd(nc, in_maps, core_ids=list(range(N_CORES)),
                               trace=TRACE, trace_cores=TRACE_CORES)
    LAST_RESULTS = res

    out = np.zeros((B, d), np.float32)
    for k in range(N_CORES):
        out_k = res.results[k]["out"]  # [sg//2, 128, 2*D] fp16
        for s, (a, idx) in enumerate(per_core[k]):
            g, j = divmod(s, NJ)
            out[idx] = out_k[g // 2, 32 * j:32 * j + len(idx),
                             (g % 2) * D:(g % 2) * D + d]
    return out


# revision 14
# speedup vs baseline: 2.6260x; 1.6410x over previous
"""Trainium2 Bass kernel for nn_AttributeOperator (MoE-style routing).

Computes out[b] = relu(attr_ops[attrs[b]] @ obj_emb[objs[b]]) for b in [0, B).

Strategy (expert-parallel, fp8, column-tiled): the dominant cost is streaming
the attr_ops table (512 x 512 x 512 fp32 = 512 MB). Samples are grouped by
attribute on the host, groups are chunked to <= 32 samples, and chunks are
load-balanced across the 8 cores (snake deal by chunk size); each core streams
only its own subset of operator matrices from HBM exactly once, cast on the
host to fp8-e3m4 scaled by OPS_SCALE (the 1/OPS_SCALE dequant is folded into
the fp16 xt). e3m4's 4 mantissa bits give rel-err 1.39e-2 vs the f32
reference (gate 2e-2); e4m3 fails at 2.3e-2, which rules out DoubleRow.

Per core the work is `sg` super-groups of 4 slots; slot rank s has a fixed
capacity cap[s] = max chunk size at that rank over all cores (SPMD: one
program fits every core's routing). The four slots of a super-group run
CONCURRENTLY in the four 32-column groups of the PE array via
tile_position=(0, 32*j), all accumulating into one [128, 512] PSUM bank (slot
j at partitions 32*j..32*j+cap). This 4x-overlaps the A-matrix streaming (the
PE-time floor is the 1 col/cycle moving-operand ingest) and lets one
full-width [128, 512] relu + one ops DMA serve 4 slots; out rows are relu'd
in fp16 into a shared buffer covering two super-groups per out-DMA. This
keeps the serial HWDGE descriptor-generation cost (~0.6 us per DMA) and the
ACT relu cost off the critical path; the ops stream owns the sync HWDGE ring
(xt/out use the scalar ring) and is the memory-roofline bottleneck.
"""

import numpy as np
import ml_dtypes

import concourse.tile as tile
from concourse import bacc, mybir
from concourse.bass_utils import run_bass_kernel_spmd

N_CORES = 8
D = 512               # embedding dim (hardcoded per problem spec)
QCH = D // 128        # contraction chunks of 128 partitions
CW = 32               # column-tile width = per-slot sample capacity
NJ = 4                # concurrent col-tiled slots per super-group
SG = 16               # minimum super-groups per core

# attr_ops values are ~N(0, 0.02); unscaled they'd land in e3m4's subnormal
# range, so scale into the normal range and fold 1/OPS_SCALE into xt.
OPS_SCALE = 64.0

# test.py hooks (ignored by the grading harness)
LAST_RESULTS = None   # BassKernelResults of the most recent run
TRACE = False
TRACE_CORES = None

OPG = 1               # super-groups per ops DMA
OW = 4                # super-groups per output window/buffer
_NC_CACHE = {}


def _build_nc(caps, ops_bufs=8, opg=None, reps=1, staggered=False):
    """Build + compile the SPMD program.

    caps[s]: sample capacity of slot rank s (s = g*NJ + j), len divisible by
    2*NJ; opg: super-groups per ops DMA; reps: HW-loop repetitions (timing).
    """
    if opg is None:
        opg = OPG
    sg = len(caps) // NJ
    ct = sum(caps)
    coff = np.concatenate([[0], np.cumsum(caps)])
    ng = -(-sg // opg)
    # output windows of OW super-groups sharing one o buffer; per window the
    # 4 col-groups are written by 4 partition-sliced DMAs of owcap rows each
    nw = -(-sg // OW)
    owcap = [max(caps[w * OW * NJ:(w + 1) * OW * NJ]) for w in range(nw)]
    owoff = np.concatenate([[0], np.cumsum([NJ * c for c in owcap])])
    tot = int(owoff[-1])
    nc = bacc.Bacc("TRN2", target_bir_lowering=False, debug=False,
                   num_devices=N_CORES)
    # ops[gg, p, (g', j, q, i)] = OPS_SCALE * A_{g,j}[i, q*128 + p] in e3m4:
    # one contiguous opg*NJ*QCH*D-byte run per partition per DMA
    ops_dram = nc.dram_tensor("ops_t", [ng, 128, opg * NJ * QCH * D],
                              mybir.dt.float8e3, kind="ExternalInput").ap()
    # xt[p, q*ct + coff[s] + c] = X_s[c, q*128 + p] / OPS_SCALE
    xt_dram = nc.dram_tensor("xt", [128, QCH * ct], mybir.dt.float16,
                             kind="ExternalInput").ap()
    # out[owoff[w] + j*owcap[w] + c, (g%OW)*D + i] for slot s=(g,j), c<cap[s]
    out_dram = nc.dram_tensor("out", [max(tot, 1), OW * D], mybir.dt.float16,
                              kind="ExternalOutput").ap()

    with tile.TileContext(nc) as tc:
        with (
            tc.tile_pool(name="xt", bufs=1) as xt_pool,
            tc.tile_pool(name="ops", bufs=ops_bufs) as ops_pool,
            tc.tile_pool(name="ps", bufs=8, space="PSUM") as ps_pool,
            tc.tile_pool(name="o", bufs=2) as o_pool,
        ):
            def body():
                xt_sb = xt_pool.tile([128, QCH * ct], mybir.dt.float16)
                nc.scalar.dma_start(xt_sb[:], xt_dram[:])

                o = None
                for g in range(sg):
                    if g % opg == 0:
                        m = ops_pool.tile([128, opg * NJ * QCH * D],
                                          mybir.dt.float8e3, tag="m")
                        nc.sync.dma_start(m[:], ops_dram[g // opg])
                    ps = ps_pool.tile([128, D], mybir.dt.float32, tag="ps")
                    for q in range(QCH):
                        for j in range(NJ):
                            s = g * NJ + j
                            cw = caps[s]
                            lhsT = xt_sb[:, q * ct + coff[s]:
                                         q * ct + coff[s] + cw]
                            rhs = m[:, (((g % opg) * NJ + j) * QCH + q) * D:
                                    (((g % opg) * NJ + j) * QCH + q + 1) * D]
                            nc.tensor.matmul(ps[32 * j:32 * j + cw, :],
                                             lhsT, rhs,
                                             start=(q == 0),
                                             stop=(q == QCH - 1),
                                             tile_position=(0, 32 * j))
                    w = g // OW
                    if g % OW == 0:
                        o = o_pool.tile([128, OW * D], mybir.dt.float16,
                                        tag="o")
                    nc.scalar.activation(
                        o[:, (g % OW) * D:(g % OW + 1) * D], ps[:],
                        mybir.ActivationFunctionType.Relu)
                    if g % OW == OW - 1 or g == sg - 1:
                        cm = owcap[w]
                        for j in range(NJ):
                            r0 = int(owoff[w]) + j * cm
                            nc.scalar.dma_start(
                                out_dram[r0:r0 + cm, :],
                                o[32 * j:32 * j + cm, :])

            if reps == 1:
                body()
            else:
                with tc.For_i(0, reps, 1,
                              hint_engines=(mybir.EngineType.PE,),
                              staggered_reset=staggered):
                    body()

    nc.compile()
    return nc


def _route(attrs):
    """Group sample indices by attribute, chunk to <= CW, snake-balance
    across cores. Returns per-core slot lists of (attr_id, idx_array),
    sorted by descending chunk size."""
    order = np.argsort(attrs, kind="stable")
    sorted_attrs = attrs[order]
    uniq, starts, counts = np.unique(sorted_attrs, return_index=True,
                                     return_counts=True)
    chunks = []
    for a, st, c in zip(uniq, starts, counts):
        idx = order[st:st + c]
        for o in range(0, c, CW):
            chunks.append((int(a), idx[o:o + CW]))
    chunks.sort(key=lambda t: -len(t[1]))
    per_core = [[] for _ in range(N_CORES)]
    for i, ch in enumerate(chunks):
        r, pos = divmod(i, N_CORES)
        k = pos if r % 2 == 0 else N_CORES - 1 - pos
        per_core[k].append(ch)
    return per_core


def _quantize_ops(attr_ops, attr_ids):
    """e3m4-quantize A^T for the given attribute ids.

    Returns {attr_id: [QCH, 128, D] e3m4 array} with [q, p, i] =
    OPS_SCALE * A[i, q*128 + p]."""
    out = {}
    ids = np.asarray(sorted(attr_ids))
    for blk in range(0, len(ids), 32):
        b = ids[blk:blk + 32]
        at = np.ascontiguousarray(
            attr_ops[b].transpose(0, 2, 1)) * np.float32(OPS_SCALE)
        q8 = at.astype(ml_dtypes.float8_e3m4).reshape(-1, QCH, 128, D)
        for i, a in enumerate(b):
            out[int(a)] = q8[i]
    return out


def _layout(per_core):
    """Per-slot-rank capacities shared by all cores, padded to 2*NJ ranks."""
    nslots = max(1, max(len(s) for s in per_core))
    sg = max(SG, -(-nslots // NJ))
    sg += sg % 2
    caps = [1] * (sg * NJ)
    for slots in per_core:
        for s, (_, idx) in enumerate(slots):
            caps[s] = max(caps[s], len(idx))
    return tuple(caps)


def _prepare(attrs, objs, attr_ops, obj_emb):
    """Route + build per-core device input maps."""
    per_core = _route(attrs)
    caps = _layout(per_core)
    sg = len(caps) // NJ
    ng = -(-sg // OPG)
    ct = sum(caps)
    coff = np.concatenate([[0], np.cumsum(caps)])

    rep = (obj_emb[objs] * np.float32(1.0 / OPS_SCALE)).astype(np.float16)
    q8 = _quantize_ops(attr_ops, {a for s in per_core for a, _ in s})
    in_maps = []
    for k in range(N_CORES):
        slots = per_core[k]
        ops_t = np.zeros((ng * OPG, 128, NJ, QCH, D), ml_dtypes.float8_e3m4)
        xt = np.zeros((128, QCH, ct), np.float16)
        for s, (a, idx) in enumerate(slots):
            g, j = divmod(s, NJ)
            ops_t[g, :, j] = q8[a].transpose(1, 0, 2)
            # xt[p, q, coff[s] + c] = rep[idx[c], q*128 + p]
            xt[:, :, coff[s]:coff[s] + len(idx)] = rep[idx].reshape(
                len(idx), QCH, 128).transpose(2, 1, 0)
        # device layout [gg, p, (g', j, q, i)]: opg consecutive super-groups
        # concatenated along the free dim
        ops_dev = np.ascontiguousarray(
            ops_t.reshape(ng, OPG, 128, NJ * QCH * D).transpose(0, 2, 1, 3)
        ).reshape(ng, 128, OPG * NJ * QCH * D)
        in_maps.append({"ops_t": ops_dev, "xt": xt.reshape(128, -1)})
    return per_core, caps, in_maps


def kernel(attrs, objs, attr_ops, obj_emb):
    global LAST_RESULTS
    attrs = np.asarray(attrs)
    objs = np.asarray(objs)
    attr_ops = np.asarray(attr_ops, dtype=np.float32)
    obj_emb = np.asarray(obj_emb, dtype=np.float32)
    B = attrs.shape[0]
    d = obj_emb.shape[1]
    assert d == D and attr_ops.shape[1:] == (D, D)

    per_core, caps, in_maps = _prepare(attrs, objs, attr_ops, obj_emb)

    nc = _NC_CACHE.get(caps)
    if nc is None:
        nc = _NC_CACHE[caps] = _build_nc(caps)

    res = run_bass_kernel_spmd(nc, in_maps, core_ids=list(range(N_CORES)),
                               trace=TRACE, trace_cores=TRACE_CORES)
    LAST_RESULTS = res

    sg = len(caps) // NJ
    nw = -(-sg // OW)
    owcap = [max(caps[w * OW * NJ:(w + 1) * OW * NJ]) for w in range(nw)]
    owoff = np.concatenate([[0], np.cumsum([NJ * c for c in owcap])])
    out = np.zeros((B, d), np.float32)
    for k in range(N_CORES):
        out_k = res.results[k]["out"]  # [tot, OW*D] fp16
        for s, (a, idx) in enumerate(per_core[k]):
            g, j = divmod(s, NJ)
            w = g // OW
            r0 = int(owoff[w]) + j * owcap[w]
            out[idx] = out_k[r0:r0 + len(idx),
                             (g % OW) * D:(g % OW) * D + d]
    return out
